# revision 2
# baseline (speedup 1.0000x reference)
"""LSNN layer forward on 8 Trainium2 NeuronCores (data-parallel over batch).

Reference math (per batch row):
    L1    = x_t @ W_syn.T + b_syn
    alpha = sigmoid((L1 + u_t) @ W_Tm.T + b_Tm)
    rho   = sigmoid((L1 + b_t) @ W_Tadp.T + b_Tadp)
    b_new = rho * b_t + (1 - rho) * spk
    thr   = 0.01 + 1.8 * b_new
    u_new = u_t + (L1 - u_t) / alpha
    o_spk = (u_new - thr > 0) as f32

Device-side formulation (all activations kept transposed, [neuron, batch]):
    1/alpha = 1 + exp(-z1)  with  z1 = (L1+u) @ W_Tm.T + b_Tm, so
    u_new - thr = t1*e + L1 - 1.8*spk - 1.8*rho*(b-spk) - 0.01
    with t1 = L1-u, e = exp(-z1).

Sharding: batch 4096 -> 8 shards of 512; weights replicated. The first
matmul runs in fp32 (exact); the two sigmoid-branch matmuls run in
float32r (TF32-like, ~1.5e-4 rel err) which only perturbs values through
a sigmoid and is 4x faster on the PE.
"""

import numpy as np

import concourse.bacc as bacc
import concourse.tile as tile
import concourse.mybir as mybir
from concourse.bass_utils import run_bass_kernel_spmd

AF = mybir.ActivationFunctionType
ALU = mybir.AluOpType

B, I, O = 4096, 2048, 2048
NCORES = 8
BC = B // NCORES          # 512 batch rows per core
P = 128                   # partitions
KT = I // P               # 16 k-tiles
OT = O // P               # 16 output neuron tiles
THR_MIN = 0.01

F32 = mybir.dt.float32
F32R = mybir.dt.float32r

# dtype of each matmul stage (mm1 = L1, mm2 = alpha branch, mm3 = rho branch)
MM1_DT = F32
MM23_DT = F32R


def build_nc():
    nc = bacc.Bacc("TRN2", target_bir_lowering=False, debug=False)

    x_d = nc.dram_tensor("x", (P, KT, BC), F32, kind="ExternalInput").ap()
    u_d = nc.dram_tensor("u", (OT, P, BC), F32, kind="ExternalInput").ap()
    b_d = nc.dram_tensor("b", (OT, P, BC), F32, kind="ExternalInput").ap()
    spk_d = nc.dram_tensor("spk", (OT, P, BC), F32, kind="ExternalInput").ap()
    wsyn_d = nc.dram_tensor("wsyn", (P, OT, KT, P), MM1_DT, kind="ExternalInput").ap()
    wtm_d = nc.dram_tensor("wtm", (P, OT, KT, P), MM23_DT, kind="ExternalInput").ap()
    wtadp_d = nc.dram_tensor("wtadp", (P, OT, KT, P), MM23_DT, kind="ExternalInput").ap()
    bsyn_d = nc.dram_tensor("bsyn", (P, OT), F32, kind="ExternalInput").ap()
    nbtm_d = nc.dram_tensor("nbtm", (P, OT), F32, kind="ExternalInput").ap()
    btadp_d = nc.dram_tensor("btadp", (P, OT), F32, kind="ExternalInput").ap()
    out_d = nc.dram_tensor("out", (OT, P, BC), F32, kind="ExternalOutput").ap()

    with tile.TileContext(nc) as tc:
        with (
            tc.tile_pool(name="persist", bufs=1) as persist,
            tc.tile_pool(name="wpool", bufs=4) as wpool,
            tc.tile_pool(name="iopool", bufs=6) as iopool,
            tc.tile_pool(name="tmp", bufs=10) as tmp,
            tc.tile_pool(name="outp", bufs=3) as outp,
            tc.tile_pool(name="psum1", bufs=2, space="PSUM") as psum1,
            tc.tile_pool(name="psum2", bufs=4, space="PSUM") as psum2,
        ):
            xsb = persist.tile([P, KT, BC], MM1_DT, tag="xsb")
            l1sb = persist.tile([P, OT, BC], F32, tag="l1sb")
            z1sb = persist.tile([P, OT, BC], MM23_DT, tag="z1sb")
            z2sb = persist.tile([P, OT, BC], MM23_DT, tag="z2sb")
            bsyn = persist.tile([P, OT], F32, tag="bsyn")
            nbtm = persist.tile([P, OT], F32, tag="nbtm")
            btadp = persist.tile([P, OT], F32, tag="btadp")

            nc.sync.dma_start(xsb[:], x_d[:])
            nc.sync.dma_start(bsyn[:], bsyn_d[:])
            nc.sync.dma_start(nbtm[:], nbtm_d[:])
            nc.sync.dma_start(btadp[:], btadp_d[:])

            # ---- phase 1: L1 = W_syn @ x (transposed), Z1 = L1+u, Z2 = L1+b
            for t in range(OT):
                w = wpool.tile([P, KT, P], MM1_DT, tag="w")
                nc.sync.dma_start(w[:], wsyn_d[:, t])
                ps = psum1.tile([P, BC], F32)
                for k in range(KT):
                    nc.tensor.matmul(ps[:], w[:, k, :], xsb[:, k, :],
                                     start=(k == 0), stop=(k == KT - 1))
                nc.scalar.activation(l1sb[:, t, :], ps[:], AF.Identity,
                                     bias=bsyn[:, t:t + 1])
                ut = iopool.tile([P, BC], F32, tag="io")
                nc.sync.dma_start(ut[:], u_d[t])
                bt = iopool.tile([P, BC], F32, tag="io")
                nc.sync.dma_start(bt[:], b_d[t])
                nc.vector.tensor_add(z1sb[:, t, :], l1sb[:, t, :], ut[:])
                nc.vector.tensor_add(z2sb[:, t, :], l1sb[:, t, :], bt[:])

            # ---- phase 2: alpha/rho branches + fused pointwise tail
            for t in range(OT):
                wa = wpool.tile([P, KT, P], MM23_DT, tag="w")
                nc.sync.dma_start(wa[:], wtm_d[:, t])
                wr = wpool.tile([P, KT, P], MM23_DT, tag="w")
                nc.sync.dma_start(wr[:], wtadp_d[:, t])
                psa = psum2.tile([P, BC], F32, tag="ps2")
                for k in range(KT):
                    nc.tensor.matmul(psa[:], wa[:, k, :], z1sb[:, k, :],
                                     start=(k == 0), stop=(k == KT - 1))
                psr = psum2.tile([P, BC], F32, tag="ps2")
                for k in range(KT):
                    nc.tensor.matmul(psr[:], wr[:, k, :], z2sb[:, k, :],
                                     start=(k == 0), stop=(k == KT - 1))

                # e = exp(-(z1 + b_Tm)) = 1/alpha - 1
                e = tmp.tile([P, BC], F32, tag="t")
                nc.scalar.activation(e[:], psa[:], AF.Exp,
                                     bias=nbtm[:, t:t + 1], scale=-1.0)
                rho = tmp.tile([P, BC], F32, tag="t")
                nc.scalar.activation(rho[:], psr[:], AF.Sigmoid,
                                     bias=btadp[:, t:t + 1])

                ut = iopool.tile([P, BC], F32, tag="io")
                nc.sync.dma_start(ut[:], u_d[t])
                bt = iopool.tile([P, BC], F32, tag="io")
                nc.sync.dma_start(bt[:], b_d[t])
                spt = iopool.tile([P, BC], F32, tag="io")
                nc.sync.dma_start(spt[:], spk_d[t])

                l1t = l1sb[:, t, :]
                t1 = tmp.tile([P, BC], F32, tag="t")
                nc.vector.tensor_sub(t1[:], l1t, ut[:])
                m = tmp.tile([P, BC], F32, tag="t")
                nc.vector.tensor_mul(m[:], t1[:], e[:])
                t2 = tmp.tile([P, BC], F32, tag="t")
                nc.vector.tensor_sub(t2[:], bt[:], spt[:])
                m2 = tmp.tile([P, BC], F32, tag="t")
                nc.vector.tensor_mul(m2[:], rho[:], t2[:])
                sp = tmp.tile([P, BC], F32, tag="t")
                nc.vector.tensor_scalar(sp[:], spt[:], -1.8, None, ALU.mult)
                s = tmp.tile([P, BC], F32, tag="t")
                nc.vector.tensor_add(s[:], l1t, sp[:])
                d1 = tmp.tile([P, BC], F32, tag="t")
                nc.vector.tensor_add(d1[:], m[:], s[:])
                m2s = tmp.tile([P, BC], F32, tag="t")
                nc.vector.tensor_scalar(m2s[:], m2[:], 1.8, None, ALU.mult)
                d = tmp.tile([P, BC], F32, tag="t")
                nc.vector.tensor_sub(d[:], d1[:], m2s[:])
                o = outp.tile([P, BC], F32, tag="o")
                nc.vector.tensor_scalar(o[:], d[:], THR_MIN, None, ALU.is_gt)
                nc.sync.dma_start(out_d[t], o[:])

    nc.compile()
    return nc


def _pack_weight(w: np.ndarray) -> np.ndarray:
    # [O, I] -> [p, o_tile, k_tile, m] with w[t*128+m, k*128+p] at [p, t, k, m]
    return np.ascontiguousarray(
        w.reshape(OT, P, KT, P).transpose(3, 0, 2, 1))


def _pack_bias(v: np.ndarray) -> np.ndarray:
    return np.ascontiguousarray(v.reshape(OT, P).T)


def prepare_in_maps(x_t, u_t, b_t, spk, W_syn, b_syn, W_Tm, b_Tm, W_Tadp, b_Tadp):
    wsyn = _pack_weight(np.asarray(W_syn, np.float32))
    wtm = _pack_weight(np.asarray(W_Tm, np.float32))
    wtadp = _pack_weight(np.asarray(W_Tadp, np.float32))
    bsyn = _pack_bias(np.asarray(b_syn, np.float32))
    nbtm = _pack_bias(-np.asarray(b_Tm, np.float32))
    btadp = _pack_bias(np.asarray(b_Tadp, np.float32))

    in_maps = []
    for c in range(NCORES):
        sl = slice(c * BC, (c + 1) * BC)
        xc = np.asarray(x_t[sl], np.float32)
        in_maps.append({
            "x": np.ascontiguousarray(xc.reshape(BC, KT, P).transpose(2, 1, 0)),
            "u": np.ascontiguousarray(
                np.asarray(u_t[sl], np.float32).reshape(BC, OT, P).transpose(1, 2, 0)),
            "b": np.ascontiguousarray(
                np.asarray(b_t[sl], np.float32).reshape(BC, OT, P).transpose(1, 2, 0)),
            "spk": np.ascontiguousarray(
                np.asarray(spk[sl], np.float32).reshape(BC, OT, P).transpose(1, 2, 0)),
            "wsyn": wsyn, "wtm": wtm, "wtadp": wtadp,
            "bsyn": bsyn, "nbtm": nbtm, "btadp": btadp,
        })
    return in_maps


def unpack_output(results) -> np.ndarray:
    # per-core out: [OT, P, BC] -> [BC, O]; concat over cores -> [B, O]
    parts = [r["out"].transpose(2, 0, 1).reshape(BC, O) for r in results]
    return np.ascontiguousarray(np.concatenate(parts, axis=0))


_NC = None


def get_nc():
    global _NC
    if _NC is None:
        _NC = build_nc()
    return _NC


def run_sharded(in_maps, trace=False, **kw):
    nc = get_nc()
    return run_bass_kernel_spmd(nc, in_maps, list(range(NCORES)), trace=trace, **kw)


def kernel(**inputs) -> np.ndarray:
    in_maps = prepare_in_maps(**inputs)
    res = run_sharded(in_maps)
    return unpack_output(res.results)


# revision 4
# speedup vs baseline: 1.5756x; 1.5756x over previous
"""LSNN layer forward on 8 Trainium2 NeuronCores (data-parallel over batch).

Reference math (per batch row):
    L1    = x_t @ W_syn.T + b_syn
    alpha = sigmoid((L1 + u_t) @ W_Tm.T + b_Tm)
    rho   = sigmoid((L1 + b_t) @ W_Tadp.T + b_Tadp)
    b_new = rho * b_t + (1 - rho) * spk
    thr   = 0.01 + 1.8 * b_new
    u_new = u_t + (L1 - u_t) / alpha
    o_spk = (u_new - thr > 0) as f32

Device-side formulation (all activations kept transposed, [neuron, batch]):
    1/alpha = 1 + exp(-z1)  with  z1 = (L1+u) @ W_Tm.T + b_Tm, so
    u_new - thr = t1*e + L1 - 1.8*spk - 1.8*rho*(b-spk) - 0.01
    with t1 = L1-u, e = exp(-z1).

Sharding: batch 4096 -> 8 shards of 512; weights replicated. The first
matmul runs in fp32 (exact); the two sigmoid-branch matmuls run in
float32r (TF32-like, ~1.5e-4 rel err) which only perturbs values through
a sigmoid and is 4x faster on the PE.
"""

import numpy as np

import concourse.bacc as bacc
import concourse.tile as tile
import concourse.mybir as mybir
from concourse.bass_utils import run_bass_kernel_spmd

AF = mybir.ActivationFunctionType
ALU = mybir.AluOpType

B, I, O = 4096, 2048, 2048
NCORES = 8
BC = B // NCORES          # 512 batch rows per core
P = 128                   # partitions
KT = I // P               # 16 k-tiles
OT = O // P               # 16 output neuron tiles
THR_MIN = 0.01

F32 = mybir.dt.float32
F32R = mybir.dt.float32r

# dtype of each matmul stage (mm1 = L1, mm2 = alpha branch, mm3 = rho branch)
import os
MM1_DT = F32R if os.environ.get('MM1_F32R') else F32
MM23_DT = F32R


def build_nc():
    nc = bacc.Bacc("TRN2", target_bir_lowering=False, debug=False)

    x_d = nc.dram_tensor("x", (P, KT, BC), MM1_DT, kind="ExternalInput").ap()
    u_d = nc.dram_tensor("u", (OT, P, BC), F32, kind="ExternalInput").ap()
    b_d = nc.dram_tensor("b", (OT, P, BC), F32, kind="ExternalInput").ap()
    spk_d = nc.dram_tensor("spk", (OT, P, BC), F32, kind="ExternalInput").ap()
    wsyn_d = nc.dram_tensor("wsyn", (P, OT, KT, P), MM1_DT, kind="ExternalInput").ap()
    wtm_d = nc.dram_tensor("wtm", (P, OT, KT, P), MM23_DT, kind="ExternalInput").ap()
    wtadp_d = nc.dram_tensor("wtadp", (P, OT, KT, P), MM23_DT, kind="ExternalInput").ap()
    bsyn_d = nc.dram_tensor("bsyn", (P, OT), F32, kind="ExternalInput").ap()
    nbtm_d = nc.dram_tensor("nbtm", (P, OT), F32, kind="ExternalInput").ap()
    btadp_d = nc.dram_tensor("btadp", (P, OT), F32, kind="ExternalInput").ap()
    out_d = nc.dram_tensor("out", (OT, P, BC), F32, kind="ExternalOutput").ap()

    with tile.TileContext(nc) as tc:
        with (
            tc.tile_pool(name="persist", bufs=1) as persist,
            tc.tile_pool(name="wpool", bufs=4) as wpool,
            tc.tile_pool(name="iopool", bufs=6) as iopool,
            tc.tile_pool(name="tmp", bufs=10) as tmp,
            tc.tile_pool(name="outp", bufs=3) as outp,
            tc.tile_pool(name="psum1", bufs=2, space="PSUM") as psum1,
            tc.tile_pool(name="psum2", bufs=4, space="PSUM") as psum2,
        ):
            xsb = persist.tile([P, KT, BC], MM1_DT, tag="xsb")
            l1sb = persist.tile([P, OT, BC], F32, tag="l1sb")
            z1sb = persist.tile([P, OT, BC], MM23_DT, tag="z1sb")
            z2sb = persist.tile([P, OT, BC], MM23_DT, tag="z2sb")
            bsyn = persist.tile([P, OT], F32, tag="bsyn")
            nbtm = persist.tile([P, OT], F32, tag="nbtm")
            btadp = persist.tile([P, OT], F32, tag="btadp")

            nc.sync.dma_start(xsb[:], x_d[:])
            nc.sync.dma_start(bsyn[:], bsyn_d[:])
            nc.sync.dma_start(nbtm[:], nbtm_d[:])
            nc.sync.dma_start(btadp[:], btadp_d[:])

            # ---- phase 1: L1 = W_syn @ x (transposed), Z1 = L1+u, Z2 = L1+b
            for t in range(OT):
                w = wpool.tile([P, KT, P], MM1_DT, tag="w")
                nc.sync.dma_start(w[:], wsyn_d[:, t])
                ps = psum1.tile([P, BC], F32)
                for k in range(KT):
                    nc.tensor.matmul(ps[:], w[:, k, :], xsb[:, k, :],
                                     start=(k == 0), stop=(k == KT - 1))
                nc.scalar.activation(l1sb[:, t, :], ps[:], AF.Identity,
                                     bias=bsyn[:, t:t + 1])
                ut = iopool.tile([P, BC], F32, tag="io")
                nc.sync.dma_start(ut[:], u_d[t])
                bt = iopool.tile([P, BC], F32, tag="io")
                nc.sync.dma_start(bt[:], b_d[t])
                nc.vector.tensor_add(z1sb[:, t, :], l1sb[:, t, :], ut[:])
                nc.vector.tensor_add(z2sb[:, t, :], l1sb[:, t, :], bt[:])

            # ---- phase 2: alpha/rho branches + fused pointwise tail
            for t in range(OT):
                wa = wpool.tile([P, KT, P], MM23_DT, tag="w")
                nc.sync.dma_start(wa[:], wtm_d[:, t])
                wr = wpool.tile([P, KT, P], MM23_DT, tag="w")
                nc.sync.dma_start(wr[:], wtadp_d[:, t])
                psa = psum2.tile([P, BC], F32, tag="ps2")
                for k in range(KT):
                    nc.tensor.matmul(psa[:], wa[:, k, :], z1sb[:, k, :],
                                     start=(k == 0), stop=(k == KT - 1))
                psr = psum2.tile([P, BC], F32, tag="ps2")
                for k in range(KT):
                    nc.tensor.matmul(psr[:], wr[:, k, :], z2sb[:, k, :],
                                     start=(k == 0), stop=(k == KT - 1))

                # e = exp(-(z1 + b_Tm)) = 1/alpha - 1
                e = tmp.tile([P, BC], F32, tag="t")
                nc.scalar.activation(e[:], psa[:], AF.Exp,
                                     bias=nbtm[:, t:t + 1], scale=-1.0)
                rho = tmp.tile([P, BC], F32, tag="t")
                nc.scalar.activation(rho[:], psr[:], AF.Sigmoid,
                                     bias=btadp[:, t:t + 1])

                ut = iopool.tile([P, BC], F32, tag="io")
                nc.sync.dma_start(ut[:], u_d[t])
                bt = iopool.tile([P, BC], F32, tag="io")
                nc.sync.dma_start(bt[:], b_d[t])
                spt = iopool.tile([P, BC], F32, tag="io")
                nc.sync.dma_start(spt[:], spk_d[t])

                l1t = l1sb[:, t, :]
                t1 = tmp.tile([P, BC], F32, tag="t")
                nc.vector.tensor_sub(t1[:], l1t, ut[:])
                m = tmp.tile([P, BC], F32, tag="t")
                nc.vector.tensor_mul(m[:], t1[:], e[:])
                t2 = tmp.tile([P, BC], F32, tag="t")
                nc.vector.tensor_sub(t2[:], bt[:], spt[:])
                m2 = tmp.tile([P, BC], F32, tag="t")
                nc.vector.tensor_mul(m2[:], rho[:], t2[:])
                sp = tmp.tile([P, BC], F32, tag="t")
                nc.vector.tensor_scalar(sp[:], spt[:], -1.8, None, ALU.mult)
                s = tmp.tile([P, BC], F32, tag="t")
                nc.vector.tensor_add(s[:], l1t, sp[:])
                d1 = tmp.tile([P, BC], F32, tag="t")
                nc.vector.tensor_add(d1[:], m[:], s[:])
                m2s = tmp.tile([P, BC], F32, tag="t")
                nc.vector.tensor_scalar(m2s[:], m2[:], 1.8, None, ALU.mult)
                d = tmp.tile([P, BC], F32, tag="t")
                nc.vector.tensor_sub(d[:], d1[:], m2s[:])
                o = outp.tile([P, BC], F32, tag="o")
                nc.vector.tensor_scalar(o[:], d[:], THR_MIN, None, ALU.is_gt)
                nc.sync.dma_start(out_d[t], o[:])

    nc.compile()
    return nc


def _pack_weight(w: np.ndarray) -> np.ndarray:
    # [O, I] -> [p, o_tile, k_tile, m] with w[t*128+m, k*128+p] at [p, t, k, m]
    return np.ascontiguousarray(
        w.reshape(OT, P, KT, P).transpose(3, 0, 2, 1))


def _pack_bias(v: np.ndarray) -> np.ndarray:
    return np.ascontiguousarray(v.reshape(OT, P).T)


def prepare_in_maps(x_t, u_t, b_t, spk, W_syn, b_syn, W_Tm, b_Tm, W_Tadp, b_Tadp):
    wsyn = _pack_weight(np.asarray(W_syn, np.float32))
    wtm = _pack_weight(np.asarray(W_Tm, np.float32))
    wtadp = _pack_weight(np.asarray(W_Tadp, np.float32))
    bsyn = _pack_bias(np.asarray(b_syn, np.float32))
    nbtm = _pack_bias(-np.asarray(b_Tm, np.float32))
    btadp = _pack_bias(np.asarray(b_Tadp, np.float32))

    in_maps = []
    for c in range(NCORES):
        sl = slice(c * BC, (c + 1) * BC)
        xc = np.asarray(x_t[sl], np.float32)
        in_maps.append({
            "x": np.ascontiguousarray(xc.reshape(BC, KT, P).transpose(2, 1, 0)),
            "u": np.ascontiguousarray(
                np.asarray(u_t[sl], np.float32).reshape(BC, OT, P).transpose(1, 2, 0)),
            "b": np.ascontiguousarray(
                np.asarray(b_t[sl], np.float32).reshape(BC, OT, P).transpose(1, 2, 0)),
            "spk": np.ascontiguousarray(
                np.asarray(spk[sl], np.float32).reshape(BC, OT, P).transpose(1, 2, 0)),
            "wsyn": wsyn, "wtm": wtm, "wtadp": wtadp,
            "bsyn": bsyn, "nbtm": nbtm, "btadp": btadp,
        })
    return in_maps


def unpack_output(results) -> np.ndarray:
    # per-core out: [OT, P, BC] -> [BC, O]; concat over cores -> [B, O]
    parts = [r["out"].transpose(2, 0, 1).reshape(BC, O) for r in results]
    return np.ascontiguousarray(np.concatenate(parts, axis=0))


_NC = None


def get_nc():
    global _NC
    if _NC is None:
        _NC = build_nc()
    return _NC


def run_sharded(in_maps, trace=False, **kw):
    nc = get_nc()
    return run_bass_kernel_spmd(nc, in_maps, list(range(NCORES)), trace=trace, **kw)


def kernel(**inputs) -> np.ndarray:
    in_maps = prepare_in_maps(**inputs)
    res = run_sharded(in_maps)
    return unpack_output(res.results)
